# revision 1
# baseline (speedup 1.0000x reference)
"""Trainium2 Bass kernel for nn_LinkEncoding (gnn_message_passing).

Takes FULL inputs (as produced by reference.setup_inputs()), shards
data-parallel over nodes across 8 NeuronCores, runs a Bass/Tile kernel,
returns the FULL [N, OUT_CH] float32 output.

Pipeline per core (slot-major layout, slot = (node, k) pair):
  host: stable-sort edges by node, keep first K per node, build a dense
        zero-padded [groups, 120, 4, 106] bf16 table of
        [t^2,t^4,t^6,t^8,t^10, edge_attr(100), 1] rows (cos() of the
        temporal encoding is folded into the head matmul via its Taylor
        series; head bias rides on the trailing 1-column).
  device: per group of 16 nodes (480 slots = 4 tiles of [120, 100]):
        gather-free sequential DMA -> PE transpose -> head matmul ->
        LN_t -> block-diag token-mix (+PSUM-accumulated residual) ->
        LN_c -> transposed channel MLP (gelu on ScalarE) -> transpose
        back with PSUM-accumulated residual -> LN_h -> matmul mean over
        K -> (per chunk of 8 groups) output projection.
All LayerNorm gains/biases are folded into adjacent matmul weights on
the host (always exact for ln_c/ln_h; for ln_t only when the params are
identity, with a general device fallback path otherwise).
"""

import math
import os
import sys

for _p in ("/opt/trn_rl_repo", "/root/.axon_site/_ro/trn_rl_repo"):
    if os.path.isdir(_p) and _p not in sys.path:
        sys.path.append(_p)

import numpy as np
import ml_dtypes

BF16 = ml_dtypes.bfloat16

# Problem constants (hardcoded per harness contract; overridable for
# small-scale debugging via module globals).
N_NODES = 50000
E_EDGES = 800000
K = 30
HID = 100
TCH = 100
OUT_CH = 100
NCORES = 8

SLOT = 4 * K          # 120 slots (4 nodes) per tile / partition dim
GW = 16               # nodes per group (4 tiles)
CH_G = 8              # groups per chunk

_CACHE = {}
LAST_RESULT = None
DEBUG_TAPS = False


def _build_nc(NG, lnt_identity):
    import concourse.bass as bass
    import concourse.tile as tile
    from concourse import bacc, mybir
    from contextlib import ExitStack

    f32 = mybir.dt.float32
    bf16 = mybir.dt.bfloat16
    AF = mybir.ActivationFunctionType
    OP = mybir.AluOpType

    nc = bacc.Bacc(None, target_bir_lowering=False)

    xin = nc.dram_tensor("xin", [NG, 106, 4 * SLOT], bf16, kind="ExternalInput")
    wht = nc.dram_tensor("wht", [106, HID], bf16, kind="ExternalInput")
    t1 = nc.dram_tensor("t1", [SLOT, 64], bf16, kind="ExternalInput")
    t2 = nc.dram_tensor("t2", [64, SLOT], bf16, kind="ExternalInput")
    w1 = nc.dram_tensor("w1", [4, HID, HID], bf16, kind="ExternalInput")
    w2 = nc.dram_tensor("w2", [4, HID, HID], bf16, kind="ExternalInput")
    owt = nc.dram_tensor("owt", [HID, HID], f32, kind="ExternalInput")
    obd = nc.dram_tensor("obd", [SLOT, 4], bf16, kind="ExternalInput")
    idb = nc.dram_tensor("idb", [128, 128], bf16, kind="ExternalInput")
    idf = nc.dram_tensor("idf", [128, 128], f32, kind="ExternalInput")
    b1 = nc.dram_tensor("b1", [HID, 4], f32, kind="ExternalInput")
    b2 = nc.dram_tensor("b2", [HID, 1], f32, kind="ExternalInput")
    ob = nc.dram_tensor("ob", [HID, 1], f32, kind="ExternalInput")
    t1b = nc.dram_tensor("t1b", [128, 1], f32, kind="ExternalInput")
    gtb = nc.dram_tensor("gtb", [SLOT, 2 * HID], f32, kind="ExternalInput")
    y = nc.dram_tensor("y", [NG * GW, HID], f32, kind="ExternalOutput")
    if DEBUG_TAPS:
        dbg0 = nc.dram_tensor("dbg0", [NG, SLOT, 4 * HID], f32, kind="ExternalOutput")
        dbg1 = nc.dram_tensor("dbg1", [NG, SLOT, 4 * HID], f32, kind="ExternalOutput")
        dbg2 = nc.dram_tensor("dbg2", [NG, SLOT, 4 * HID], f32, kind="ExternalOutput")
        dbg3 = nc.dram_tensor("dbg3", [NG, SLOT, 4 * HID], f32, kind="ExternalOutput")
        dbg4 = nc.dram_tensor("dbg4", [NG, SLOT, 4 * HID], f32, kind="ExternalOutput")
        dbg5 = nc.dram_tensor("dbg5", [NG, 64, 4 * HID], f32, kind="ExternalOutput")
        dbg6 = nc.dram_tensor("dbg6", [NG, SLOT, 16], f32, kind="ExternalOutput")

    with tile.TileContext(nc) as tc, ExitStack() as ctx:
        singles = ctx.enter_context(tc.tile_pool(name="singles", bufs=1))
        pg = ctx.enter_context(tc.tile_pool(name="pg", bufs=3))
        pgt = ctx.enter_context(tc.tile_pool(name="pgt", bufs=2))
        pz = ctx.enter_context(tc.tile_pool(name="pz", bufs=2))
        pgel = ctx.enter_context(tc.tile_pool(name="pgel", bufs=2))
        pzt = ctx.enter_context(tc.tile_pool(name="pzt", bufs=2))
        pgh = ctx.enter_context(tc.tile_pool(name="pgh", bufs=2))
        phc = ctx.enter_context(tc.tile_pool(name="phc", bufs=2))
        pstat = ctx.enter_context(tc.tile_pool(name="pstat", bufs=3))
        ptm = ctx.enter_context(tc.tile_pool(name="ptm", bufs=2))
        pob = ctx.enter_context(tc.tile_pool(name="pob", bufs=2))
        # PSUM pools: 2+2+2+2 = 8 banks total
        pbx = ctx.enter_context(tc.tile_pool(name="pbx", bufs=2, space="PSUM"))
        pps = ctx.enter_context(tc.tile_pool(name="pps", bufs=2, space="PSUM"))
        pch = ctx.enter_context(tc.tile_pool(name="pch", bufs=2, space="PSUM"))
        pm = ctx.enter_context(tc.tile_pool(name="pm", bufs=2, space="PSUM"))

        # constants
        s_wht = singles.tile([106, HID], bf16)
        nc.sync.dma_start(s_wht[:], wht[:, :])
        s_t1 = singles.tile([SLOT, 64], bf16)
        nc.sync.dma_start(s_t1[:], t1[:, :])
        s_t2 = singles.tile([64, SLOT], bf16)
        nc.sync.dma_start(s_t2[:], t2[:, :])
        s_w1 = singles.tile([HID, 4, HID], bf16)
        nc.sync.dma_start(s_w1[:], w1[:, :, :].rearrange("j c o -> c j o"))
        s_w2 = singles.tile([HID, 4, HID], bf16)
        nc.sync.dma_start(s_w2[:], w2[:, :, :].rearrange("j c o -> c j o"))
        s_owt = singles.tile([HID, HID], f32)
        nc.sync.dma_start(s_owt[:], owt[:, :])
        s_obd = singles.tile([SLOT, 4], bf16)
        nc.sync.dma_start(s_obd[:], obd[:, :])
        s_idb = singles.tile([128, 128], bf16)
        nc.sync.dma_start(s_idb[:], idb[:, :])
        s_idf = singles.tile([128, 128], f32)
        nc.sync.dma_start(s_idf[:], idf[:, :])
        s_b1 = singles.tile([HID, 4], f32)
        nc.sync.dma_start(s_b1[:], b1[:, :])
        s_b2 = singles.tile([HID, 1], f32)
        nc.sync.dma_start(s_b2[:], b2[:, :])
        s_ob = singles.tile([HID, 1], f32)
        nc.sync.dma_start(s_ob[:], ob[:, :])
        s_t1b = singles.tile([128, 1], f32)
        nc.sync.dma_start(s_t1b[:], t1b[:, :])
        s_gtb = singles.tile([SLOT, 2 * HID], f32)
        nc.sync.dma_start(s_gtb[:], gtb[:, :])
        s_eps = singles.tile([128, 1], f32)
        nc.vector.memset(s_eps[:], 1e-5)

        def emit_ln(src, apply_gtb=False):
            """src: [SLOT, 4*HID] f32 PSUM tile. Returns z bf16 [SLOT,4,HID]."""
            srcv = src.rearrange("p (g c) -> p g c", g=4)
            st = pstat.tile([SLOT, 4, 6], f32, tag="st")
            for t in range(4):
                nc.vector.bn_stats(st[:, t, :], srcv[:, t, :])
            mv = pstat.tile([SLOT, 4, 2], f32, tag="mv")
            for t in range(4):
                nc.vector.bn_aggr(mv[:, t, :], st[:, t, :])
            sd = pstat.tile([SLOT, 4], f32, tag="sd")
            nc.scalar.activation(sd[:], mv[:, :, 1], AF.Sqrt,
                                 bias=s_eps[:SLOT, :], scale=1.0)
            ri = pstat.tile([SLOT, 4], f32, tag="ri")
            nc.vector.reciprocal(ri[:], sd[:])
            nma = pstat.tile([SLOT, 4], f32, tag="nma")
            nc.vector.tensor_scalar_mul(nma[:], mv[:, :, 0], -1.0)
            nm = pstat.tile([SLOT, 4], f32, tag="nm")
            nc.vector.tensor_tensor(nm[:], nma[:], ri[:], op=OP.mult)
            z = pz.tile([SLOT, 4, HID], bf16,
                        tag="zg" if apply_gtb else "z")
            for t in range(4):
                nc.scalar.activation(z[:, t, :], src[:, t * HID:(t + 1) * HID],
                                     AF.Identity, bias=nm[:, t:t + 1],
                                     scale=ri[:, t:t + 1])
            emit_ln.last_stats = (mv, ri, nm)
            if apply_gtb and not lnt_identity:
                # general ln_t: z = z*g + b with host-tiled [SLOT, HID] g/b
                import concourse.bass as bass_mod
                gsl = s_gtb[:, 0:HID]
                bsl = s_gtb[:, HID:2 * HID]
                gbc = bass_mod.AP(tensor=gsl.tensor, offset=gsl.offset,
                                  ap=[gsl.ap[0], [0, 4], gsl.ap[1]])
                bbc = bass_mod.AP(tensor=bsl.tensor, offset=bsl.offset,
                                  ap=[bsl.ap[0], [0, 4], bsl.ap[1]])
                z2 = pz.tile([SLOT, 4, HID], bf16, tag="z2")
                nc.vector.tensor_tensor(z2[:], z[:], gbc, op=OP.mult)
                nc.vector.tensor_tensor(z[:], z2[:], bbc, op=OP.add)
            return z

        nchunks = (NG + CH_G - 1) // CH_G

        for ci in range(nchunks):
            g0 = ci * CH_G
            gn = min(CH_G, NG - g0)
            TM = pm.tile([HID, GW * gn], f32, tag="tm")
            for gi in range(gn):
                g = g0 + gi
                GTs = pgt.tile([106, 4 * SLOT], bf16, tag="gt")
                nc.sync.dma_start(GTs[:], xin[g, :, :])
                Bx = pbx.tile([SLOT, 4 * HID], f32, tag="bx")
                for t in range(4):
                    nc.tensor.matmul(Bx[:, t * HID:(t + 1) * HID],
                                     GTs[:, t * SLOT:(t + 1) * SLOT],
                                     s_wht[:], start=(t == 0), stop=True,
                                     skip_group_check=True)
                if DEBUG_TAPS:
                    _d = pob.tile([SLOT, 4 * HID], f32, tag="dbg")
                    nc.vector.tensor_copy(_d[:], Bx[:])
                    nc.sync.dma_start(dbg0[g, :, :], _d[:])
                z = emit_ln(Bx, apply_gtb=True)
                mvX, riX, nmX = emit_ln.last_stats
                if DEBUG_TAPS:
                    _d = pob.tile([SLOT, 4 * HID], f32, tag="dbg")
                    nc.vector.tensor_copy(_d[:], z.rearrange("p g c -> p (g c)"))
                    nc.sync.dma_start(dbg4[g, :, :], _d[:])
                    _d6 = pob.tile([SLOT, 16], f32, tag="dbg6")
                    nc.vector.tensor_copy(_d6[:, 0:4], mvX[:, :, 0])
                    nc.vector.tensor_copy(_d6[:, 4:8], mvX[:, :, 1])
                    nc.vector.tensor_copy(_d6[:, 8:12], riX[:])
                    nc.vector.tensor_copy(_d6[:, 12:16], nmX[:])
                    nc.sync.dma_start(dbg6[g, :, :], _d6[:])
                Bt = pps.tile([64, 4 * HID], f32, tag="ps")
                for t in range(4):
                    nc.tensor.matmul(
                        Bt[:, HID * t:HID * t + HID],
                        s_t1[:], z[:, t, :], start=(t == 0), stop=True,
                        skip_group_check=True)
                gel = pgel.tile([64, 4 * HID], bf16, tag="gel")
                nc.scalar.activation(gel[:], Bt[:], AF.Gelu,
                                     bias=s_t1b[:64, 0:1], scale=1.0)
                if DEBUG_TAPS:
                    _d5 = pob.tile([64, 4 * HID], f32, tag="dbg5")
                    nc.vector.tensor_copy(_d5[:], gel[:])
                    nc.sync.dma_start(dbg5[g, :, :], _d5[:])
                for t in range(4):
                    nc.tensor.matmul(
                        Bx[:, t * HID:(t + 1) * HID], s_t2[:],
                        gel[:, HID * t:HID * t + HID],
                        start=False, stop=True, skip_group_check=True)
                if DEBUG_TAPS:
                    _d = pob.tile([SLOT, 4 * HID], f32, tag="dbg")
                    nc.vector.tensor_copy(_d[:], Bx[:])
                    nc.sync.dma_start(dbg1[g, :, :], _d[:])
                zc = emit_ln(Bx)
                ZT = pps.tile([HID, 4 * SLOT], bf16, tag="ps")
                for t in range(4):
                    nc.tensor.matmul(ZT[:, t * SLOT:(t + 1) * SLOT],
                                     zc[:, t, :], s_idb[:SLOT, :SLOT],
                                     is_transpose=True, start=(t == 0),
                                     stop=True, skip_group_check=True)
                zcT = pzt.tile([HID, 4 * SLOT], bf16, tag="zt")
                nc.vector.tensor_copy(zcT[:], ZT[:])
                gh = pgh.tile([HID, 4, 4 * SLOT], bf16, tag="gh")
                for j in range(4):
                    Bh = pch.tile([HID, 4 * SLOT], f32, tag="bh")
                    nc.tensor.matmul(Bh[:], s_w1[:, j, :], zcT[:],
                                     start=True, stop=True)
                    nc.scalar.activation(gh[:, j, :], Bh[:], AF.Gelu,
                                         bias=s_b1[:, j:j + 1], scale=1.0)
                Bc = pps.tile([HID, 4 * SLOT], f32, tag="ps")
                for j in range(4):
                    nc.tensor.matmul(Bc[:], s_w2[:, j, :], gh[:, j, :],
                                     start=(j == 0), stop=(j == 3))
                hcT = phc.tile([HID, 4 * SLOT], f32, tag="hc")
                nc.scalar.activation(hcT[:], Bc[:], AF.Identity,
                                     bias=s_b2[:, 0:1], scale=1.0)
                for t in range(4):
                    nc.tensor.matmul(Bx[:, t * HID:(t + 1) * HID],
                                     hcT[:, t * SLOT:(t + 1) * SLOT],
                                     s_idf[:HID, :HID], is_transpose=True,
                                     start=False, stop=True,
                                     skip_group_check=True)
                if DEBUG_TAPS:
                    _d = pob.tile([SLOT, 4 * HID], f32, tag="dbg")
                    nc.vector.tensor_copy(_d[:], Bx[:])
                    nc.sync.dma_start(dbg2[g, :, :], _d[:])
                zh = emit_ln(Bx)
                if DEBUG_TAPS:
                    _d = pob.tile([SLOT, 4 * HID], f32, tag="dbg")
                    nc.vector.tensor_copy(_d[:], zh.rearrange("p g c -> p (g c)"))
                    nc.sync.dma_start(dbg3[g, :, :], _d[:])
                for t in range(4):
                    nc.tensor.matmul(TM[:, 4 * (4 * gi + t):4 * (4 * gi + t) + 4],
                                     zh[:, t, :], s_obd[:],
                                     start=(gi == 0 and t == 0), stop=True,
                                     skip_group_check=True)
            # chunk finale: output projection for GW*gn nodes
            nn = GW * gn
            tm_s = ptm.tile([HID, GW * CH_G], f32, tag="tms")
            nc.vector.tensor_copy(tm_s[:, :nn], TM[:])
            P2 = pps.tile([HID, GW * CH_G], f32, tag="ps")
            nc.tensor.matmul(P2[:, :nn], s_owt[:], tm_s[:, :nn],
                             start=True, stop=True)
            pj = ptm.tile([HID, GW * CH_G], f32, tag="pj")
            nc.scalar.activation(pj[:, :nn], P2[:, :nn], AF.Identity,
                                 bias=s_ob[:, 0:1], scale=1.0)
            PF = pps.tile([GW * CH_G, HID], f32, tag="ps")
            nc.tensor.matmul(PF[:nn, :], pj[:, :nn], s_idf[:HID, :HID],
                             is_transpose=True, start=True, stop=True)
            ob_s = pob.tile([GW * CH_G, HID], f32, tag="ob")
            nc.vector.tensor_copy(ob_s[:nn, :], PF[:nn, :])
            nc.sync.dma_start(y[g0 * GW:g0 * GW + nn, :], ob_s[:nn, :])
    nc.compile()
    return nc


def _host_prepare(inputs):
    """Build per-core device input maps from the full problem inputs."""
    ea = np.asarray(inputs["edge_attr"], dtype=np.float32)
    et = np.asarray(inputs["edge_time"], dtype=np.float32)
    nb = np.asarray(inputs["node_batch"]).astype(np.int64)
    N = int(np.asarray(inputs["num_nodes"]))
    E = nb.shape[0]

    head_w = np.asarray(inputs["head_w"], dtype=np.float64)
    head_b = np.asarray(inputs["head_b"], dtype=np.float64)
    ln_t_g = np.asarray(inputs["ln_t_g"], dtype=np.float64)
    ln_t_b = np.asarray(inputs["ln_t_b"], dtype=np.float64)
    tok1_w = np.asarray(inputs["tok1_w"], dtype=np.float64)
    tok1_b = np.asarray(inputs["tok1_b"], dtype=np.float64)
    tok2_w = np.asarray(inputs["tok2_w"], dtype=np.float64)
    tok2_b = np.asarray(inputs["tok2_b"], dtype=np.float64)
    ln_c_g = np.asarray(inputs["ln_c_g"], dtype=np.float64)
    ln_c_b = np.asarray(inputs["ln_c_b"], dtype=np.float64)
    ch1_w = np.asarray(inputs["ch1_w"], dtype=np.float64)
    ch1_b = np.asarray(inputs["ch1_b"], dtype=np.float64)
    ch2_w = np.asarray(inputs["ch2_w"], dtype=np.float64)
    ch2_b = np.asarray(inputs["ch2_b"], dtype=np.float64)
    ln_h_g = np.asarray(inputs["ln_h_g"], dtype=np.float64)
    ln_h_b = np.asarray(inputs["ln_h_b"], dtype=np.float64)
    out_w = np.asarray(inputs["out_w"], dtype=np.float64)
    out_b = np.asarray(inputs["out_b"], dtype=np.float64)

    NPC = (N + NCORES - 1) // NCORES          # nodes per core
    NPCP = ((NPC + GW - 1) // GW) * GW        # padded to group multiple
    NG = NPCP // GW

    # --- edge -> slot assignment (stable sort, first K per node) ---
    order = np.argsort(nb, kind="stable")
    snb = nb[order]
    pos = np.arange(E, dtype=np.int64) - np.searchsorted(snb, snb, side="left")
    keep = pos < K
    le = order[keep]                 # edge ids, slot-ordered
    lnode = snb[keep]
    lk = pos[keep]
    core = (lnode // NPC).astype(np.int64)
    nl = (lnode % NPC).astype(np.int64)

    # --- dense slot table [cores, NPCP, K, 106] bf16 ---
    dense = np.zeros((NCORES, NPCP, K, 106), dtype=BF16)
    t64 = et[le].astype(np.float64)
    t2 = t64 * t64
    tp = np.stack([t2, t2 ** 2, t2 ** 3, t2 ** 4, t2 ** 5], axis=1)
    dense[core, nl, lk, 0:5] = tp.astype(np.float32)
    dense[core, nl, lk, 5:105] = ea[le]
    dense[core, nl, lk, 105] = np.float32(1.0)

    # --- folded weights ---
    sqrt_d = math.sqrt(TCH)
    tw = 1.0 / sqrt_d ** np.linspace(0.0, sqrt_d, TCH)  # float64
    W_time = head_w[:, :TCH]
    W_attr = head_w[:, TCH:]
    C = []
    for m in range(6):
        coef = ((-1.0) ** m) / math.factorial(2 * m)
        C.append(coef * (W_time @ (tw ** (2 * m))))     # [HID]
    wht = np.zeros((106, HID), dtype=np.float32)
    for m in range(1, 6):
        wht[m - 1, :] = C[m]
    wht[5:105, :] = W_attr.T
    wht[105, :] = head_b + C[0]

    lnt_identity = bool(np.allclose(ln_t_g, 1.0) and np.allclose(ln_t_b, 0.0))

    t1m = np.zeros((SLOT, 64), dtype=np.float32)
    t2m = np.zeros((64, SLOT), dtype=np.float32)
    for b in range(4):
        t1m[30 * b:30 * b + 30, 16 * b:16 * b + 15] = tok1_w.T
        t2m[16 * b:16 * b + 15, 30 * b:30 * b + 30] = tok2_w.T
    t1bv = np.zeros((128, 1), dtype=np.float32)
    for b in range(8):
        t1bv[16 * b:16 * b + 15, 0] = tok1_b
    # tok2_b dropped: constant per-slot shift is invariant under LN_c /
    # LN_h (which are the only consumers of h_token / h_channel).

    Wg1 = ch1_w * ln_c_g[None, :]
    b1p = ch1_b + ch1_w @ ln_c_b
    w1m = np.stack([Wg1[HID * j:HID * (j + 1), :].T for j in range(4)])
    b1m = np.stack([b1p[HID * j:HID * (j + 1)] for j in range(4)], axis=1)
    w2m = np.stack([ch2_w[:, HID * j:HID * (j + 1)].T for j in range(4)])
    b2m = ch2_b[:, None]

    OWg = out_w * ln_h_g[None, :]
    owtm = OWg.T
    obm = (out_b + out_w @ ln_h_b)[:, None]

    obdm = np.zeros((SLOT, 4), dtype=np.float32)
    for b in range(4):
        obdm[30 * b:30 * b + 30, b] = 1.0 / K

    gtbm = np.zeros((SLOT, 2 * HID), dtype=np.float32)
    gtbm[:, :HID] = ln_t_g[None, :]
    gtbm[:, HID:] = ln_t_b[None, :]

    ident = np.eye(128, dtype=np.float32)

    base = {
        "wht": wht.astype(BF16),
        "t1": t1m.astype(BF16),
        "t2": t2m.astype(BF16),
        "w1": w1m.astype(BF16),
        "w2": w2m.astype(BF16),
        "owt": owtm.astype(np.float32),
        "obd": obdm.astype(BF16),
        "idb": ident.astype(BF16),
        "idf": ident,
        "b1": b1m.astype(np.float32),
        "b2": b2m.astype(np.float32),
        "ob": obm.astype(np.float32),
        "t1b": t1bv,
        "gtb": gtbm,
    }

    in_maps = []
    for c in range(NCORES):
        d = dense[c].reshape(NG, 4, 4, K, 106)       # [g, t, u, k, c]
        # pre-transposed: [g, feature, t, u*k] so the head matmul's lhsT
        # (GT) comes straight from DMA with no PE transpose
        d = np.ascontiguousarray(d.transpose(0, 4, 1, 2, 3))  # [g, c, t, u, k]
        d = d.reshape(NG, 106, 4 * SLOT)
        m = dict(base)
        m["xin"] = d
        in_maps.append(m)
    return in_maps, NG, NPC, NPCP, lnt_identity, N


def kernel(**inputs):
    global LAST_RESULT
    from concourse.bass_utils import run_bass_kernel_spmd

    in_maps, NG, NPC, NPCP, lnt_identity, N = _host_prepare(inputs)

    key = (NG, lnt_identity, DEBUG_TAPS)
    if key not in _CACHE:
        _CACHE[key] = _build_nc(NG, lnt_identity)
    nc = _CACHE[key]

    res = run_bass_kernel_spmd(nc, in_maps, core_ids=list(range(NCORES)))
    LAST_RESULT = res

    parts = []
    remaining = N
    for c in range(NCORES):
        take = min(NPC, remaining)
        parts.append(res.results[c]["y"][:take])
        remaining -= take
    out = np.concatenate(parts, axis=0).astype(np.float32)
    return out

